# revision 1
# baseline (speedup 1.0000x reference)
"""Trainium2 Bass kernel for nn_BusDecoder (moe_routing).

Computes out[b, n*2+o] = sum_d H[b,n,d] * W[t_n, d, o] + b[t_n, o] with
t_n = bus_type[0, n], for B=32, N=4096, D=1024, OUT=2, 3 types.

Strategy (memory-bound regime):
  - Data-parallel over batch B across 8 cores (B_local=4 per core).
  - Host splits H into a precision-stacked stream (Dekker-style) transposed to
    feature-major: f16 hi plus a lo residual (f16, or f8e4m3 scaled by 2^9 in
    the default "f8lo" mode); the device matmuls recover near-fp32 accuracy:
        out = Hhi@Whi + Hhi@Wlo + Hlo@Whi (+bias)
    (measured absmax-rel err: 9e-7 with f16 lo, 8e-6 with f8 lo, 2e-4 with
    no lo stream — select via KERNEL_PREC = f16lo | f8lo | none).
  - H is pre-tiled on the host to the exact per-chunk SBUF layout so every
    chunk DMA reads one contiguous run per partition (~366 GB/s vs ~275 GB/s
    for a naive feature-major layout — this is the main memory-rate lever).
  - On device, one CSTK-wide weight stack [Whi | Wlo | pad] is the stationary
    operand; Hhi streams once (producing both Whi and Wlo products), the lo
    stream accumulates into the same PSUM bank (f16 lo into partitions 0:6;
    f8 lo into its own zero-padded 12:18 block, descaled via the mask).
  - Bias is folded into the VectorE select as a per-partition f32 scalar add.
  - Routing/selection on device: PSUM[CSTK, F] * mask (one-hot by bus type,
    per token) on VectorE, then a K=CSTK matmul with a constant 0/1 matrix
    T[CSTK, 2] sums the per-type pair into the final [2, F] output (exact in
    fp32 since the weights are 0/1). The select stage for group g is emitted
    one group late so the PE never stalls waiting on the VectorE multiply;
    output stores go on the nc.scalar HWDGE ring so they never block loads.
"""

import os

import numpy as np

import concourse.bacc as bacc
import concourse.bass_utils as bass_utils
import concourse.mybir as mybir
import concourse.tile as tile

B, N, D, OUT = 32, 4096, 1024, 2
N_TYPES = 3
N_CORES = 8
BL = B // N_CORES          # 4 batch rows per core
TOK = BL * N               # 16384 tokens per core
P = 128
DCH = D // P               # 8 contraction chunks
CH = 1024                  # tokens per DMA chunk (4 MiB per combined chunk)
G = 512                    # tokens per matmul group (one PSUM bank of fp32)

# Precision of the H stream (override via env for experiments):
#   f8lo:  f16 hi + f8e4m3 lo scaled by 2^9 (48 MiB/core, ~143 us, 8e-6 err)
#   f16lo: f16 hi + f16 lo (64 MiB/core, ~190 us, 9e-7 err)
#   none:  f16 only (32 MiB/core, ~83 us, 2e-4 err)
PREC = os.environ.get("KERNEL_PREC", "f8lo")
USE_LO = PREC == "f16lo"
USE_F8 = PREC == "f8lo"
NHP = 2 * DCH if USE_LO else DCH   # h-chunk sub-tiles (hi block + lo block)
CSTK = 18 if USE_F8 else 12        # stationary stack width
C12 = CSTK                         # stack width alias used in shapes below
F8_SCALE = 512.0                   # lo is stored as f8(lo * F8_SCALE)

_CACHED_NC = {}


def _build_nc(repeat=1, ch=CH, hbufs=3, split_dma=True, use_selbuf=False,
              mode="full"):
    # repeat>1 wraps the body in a device-side For_i loop running the
    # identical workload `repeat` times — used only by test.py to measure
    # per-execution hardware time through the high-latency axon tunnel.
    # mode: "full" | "dma" (loads only) | "compute" (loads once, loops math)
    key = (repeat, ch, hbufs, split_dma, use_selbuf, mode)
    if key in _CACHED_NC:
        return _CACHED_NC[key]

    f16 = mybir.dt.float16
    f32 = mybir.dt.float32

    nc = bacc.Bacc("TRN2", debug=False)
    # h2 is host-pre-tiled to the exact per-chunk SBUF layout so each chunk
    # DMA reads one contiguous 32 KB run per partition (measured ~366 GB/s
    # vs ~275 GB/s for the naive feature-major layout):
    #   h2[c, p, a*DCH+do, t] = part_a(H^T)[do*128+p, c*CH+t]
    assert ch == CH, "h2 DRAM layout is pre-tiled for the default CH"
    h2 = nc.dram_tensor("h2", [TOK // CH, P, NHP, CH], f16,
                        kind="ExternalInput")
    if USE_F8:
        f8 = mybir.dt.float8e4
        h8 = nc.dram_tensor("h8", [TOK // CH, P, DCH, CH], f8,
                            kind="ExternalInput")
        wstk8 = nc.dram_tensor("wstk8", [D, CSTK], f8, kind="ExternalInput")
    wstk = nc.dram_tensor("wstk", [D, C12], f16, kind="ExternalInput")
    bvec = nc.dram_tensor("bvec", [C12, 1], f32, kind="ExternalInput")
    mask = nc.dram_tensor("mask12", [C12, TOK], f32, kind="ExternalInput")
    tmat = nc.dram_tensor("tmat", [C12, OUT], f16, kind="ExternalInput")
    out = nc.dram_tensor("out", [OUT, TOK], f32, kind="ExternalOutput")

    with tile.TileContext(nc) as tc:
        with (
            tc.tile_pool(name="const", bufs=1) as cp,
            tc.tile_pool(name="hp", bufs=hbufs) as hp,
            tc.tile_pool(name="wk", bufs=3) as wk,
            tc.tile_pool(name="ps", bufs=3, space="PSUM") as ps,
            tc.tile_pool(name="ps2", bufs=2, space="PSUM") as ps2,
        ):
            wt = cp.tile([P, DCH, C12], f16, name="wt")
            nc.sync.dma_start(wt[:], wstk.ap().rearrange("(do p) c -> p do c", p=P))
            if USE_F8:
                wt8 = cp.tile([P, DCH, CSTK], mybir.dt.float8e4, name="wt8")
                nc.sync.dma_start(
                    wt8[:], wstk8.ap().rearrange("(do p) c -> p do c", p=P))
            else:
                wt8 = None
            bv = cp.tile([C12, 1], f32, name="bv")
            nc.sync.dma_start(bv[:], bvec.ap())
            tt = cp.tile([C12, OUT], f16, name="tt")
            nc.sync.dma_start(tt[:], tmat.ap())
            # mask rides the scalar HWDGE ring: it is slow (18 partitions ->
            # few DMA ports) and on the sync ring it would delay the first
            # H-chunk loads (FIFO per ring)
            msk = cp.tile([C12, TOK], f32, name="msk")
            nc.scalar.dma_start(msk[:], mask.ap())
            selbuf = cp.tile([OUT, TOK], f32, name="selbuf") if use_selbuf else None

            hv = h2.ap()
            hv8 = h8.ap() if USE_F8 else None

            def body():
                _emit_body(nc, hv, out, hp, wk, ps, ps2, wt, bv, tt, msk,
                           ch, split_dma, selbuf, mode, hv8, wt8)

            if repeat == 1:
                body()
            else:
                with tc.For_i(0, repeat, 1):
                    body()

    nc.compile()
    _CACHED_NC[key] = nc
    return nc


def _emit_body(nc, hv, out, hp, wk, ps, ps2, wt, bv, tt, msk,
               ch, split_dma, selbuf, mode="full", hv8=None, wt8=None):
    f16 = mybir.dt.float16
    f32 = mybir.dt.float32

    def emit_main(ht, g, ht8=None):
        gs = slice(g * G, (g + 1) * G)
        p = ps.tile([C12, G], f32, name="p")
        last_hi = not (USE_LO or USE_F8)
        for do in range(DCH):
            nc.tensor.matmul(
                p[:], wt[:, do, :], ht[:, do, gs],
                start=(do == 0), stop=(last_hi and do == DCH - 1),
                skip_group_check=True,
            )
        if USE_LO:
            for do in range(DCH):
                nc.tensor.matmul(
                    p[0:6], wt[:, do, 0:6], ht[:, DCH + do, gs],
                    start=False, stop=(do == DCH - 1), skip_group_check=True,
                )
        if USE_F8:
            for do in range(DCH):
                nc.tensor.matmul(
                    p[:], wt8[:, do, :], ht8[:, do, gs],
                    start=False, stop=(do == DCH - 1), skip_group_check=True,
                )
        return p

    def emit_select(p, off):
        # m = (p + bias) * mask, then Dekker-split m to f16 hi/lo so the
        # pair-sum runs as two exact f16 matmuls (1 cyc/row) instead of one
        # fp32 matmul (4 cyc/row).
        m = wk.tile([C12, G], f32, name="m")
        nc.vector.scalar_tensor_tensor(
            m[:], p[:], bv[:, 0:1], msk[:, off:off + G],
            mybir.AluOpType.add, mybir.AluOpType.mult,
        )
        mhi = wk.tile([C12, G], f16, name="mhi")
        nc.vector.tensor_copy(mhi[:], m[:])
        mlo = wk.tile([C12, G], f16, name="mlo")
        nc.vector.tensor_sub(mlo[:], m[:], mhi[:])
        p2 = ps2.tile([OUT, G], f32, name="p2")
        nc.tensor.matmul(
            p2[:], tt[:], mhi[:], start=True, stop=False, skip_group_check=True,
        )
        nc.tensor.matmul(
            p2[:], tt[:], mlo[:], start=False, stop=True, skip_group_check=True,
        )
        if selbuf is not None:
            nc.vector.tensor_copy(selbuf[:, off:off + G], p2[:])
        else:
            sg = wk.tile([OUT, G], f32, name="sg")
            nc.vector.tensor_copy(sg[:], p2[:])
            nc.scalar.dma_start(out.ap()[:, off:off + G], sg[:])

    if mode == "compute":
        ht0 = hp.tile([P, NHP, ch], f16, name="ht", bufs=1)
        nc.sync.dma_start(ht0[:], hv[0])
        pending = None
        for c in range(TOK // ch):
            for g in range(ch // G):
                p = emit_main(ht0, g)
                if pending is not None:
                    emit_select(*pending)
                pending = (p, c * ch + g * G)
        emit_select(*pending)
        return

    pending = None
    for c in range(TOK // ch):
        ht = hp.tile([P, NHP, ch], f16, name="ht")
        if split_dma:
            nc.sync.dma_start(ht[:, :DCH], hv[c, :, :DCH])
            if USE_LO:
                nc.sync.dma_start(ht[:, DCH:], hv[c, :, DCH:])
        else:
            nc.sync.dma_start(ht[:], hv[c])
        if USE_F8:
            ht8 = hp.tile([P, DCH, ch], mybir.dt.float8e4, name="ht8")
            nc.sync.dma_start(ht8[:], hv8[c])
        else:
            ht8 = None
        if mode == "dma":
            # keep a reader so buffers recycle without stalling the queue
            nc.vector.tensor_copy(msk[0:1, 0:8], ht[0:1, 0, 0:8])
            continue
        for g in range(ch // G):
            p = emit_main(ht, g, ht8)
            if pending is not None:
                emit_select(*pending)
            pending = (p, c * ch + g * G)
    if mode == "dma":
        return
    emit_select(*pending)
    if selbuf is not None:
        nc.sync.dma_start(out.ap(), selbuf[:])


def _host_prep(H, bus_type, W, b):
    """Shard + precision-split inputs; returns per-core in_maps."""
    H = np.asarray(H, dtype=np.float32)
    W = np.asarray(W, dtype=np.float32)
    b = np.asarray(b, dtype=np.float32)
    types = np.asarray(bus_type)[0].astype(np.int64)  # decoder choice = row 0

    # Weight stack [D, CSTK]: cols 2t+o = Whi[t,:,o], cols 6+2t+o = Wlo[t,:,o]
    # (+ 6 zero cols for the f8-lo block when PREC == "f8lo")
    W6 = np.ascontiguousarray(W.transpose(1, 0, 2).reshape(D, 2 * N_TYPES))
    Whi = W6.astype(np.float16)
    Wlo = (W6 - Whi.astype(np.float32)).astype(np.float16)
    zpad = np.zeros((D, CSTK - 12), np.float16)
    wstk = np.ascontiguousarray(np.concatenate([Whi, Wlo, zpad], axis=1))

    # Exact f32 bias, applied per-partition on VectorE before the mask-mul
    bvec = np.zeros((C12, 1), np.float32)
    bvec[0:2 * N_TYPES, 0] = b.reshape(2 * N_TYPES)

    # One-hot routing mask per token (token j = b_local*N + n -> depends on n)
    oh = (types[None, :] == np.arange(N_TYPES)[:, None])      # [3, N]
    m6 = np.repeat(oh, 2, axis=0)                             # [6, N]
    m6t = np.tile(m6, (1, BL)).astype(np.float32)             # [6, TOK]
    blocks = [m6t, m6t]
    if USE_F8:
        blocks.append(m6t / F8_SCALE)  # undo the f8 lo-part scaling
    mask12 = np.ascontiguousarray(np.concatenate(blocks, axis=0))

    # Constant pair-sum matrix: sel[o] = sum_{c: c%2==o} M[c] (exact in f16)
    tmat = np.zeros((C12, OUT), np.float16)
    tmat[0::2, 0] = 1.0
    tmat[1::2, 1] = 1.0

    if USE_F8:
        f8dt = mybir.dt.np(mybir.dt.float8e4)
        wstk8 = np.zeros((D, CSTK), f8dt)
        wstk8[:, 12:18] = Whi.astype(np.float32).astype(f8dt)

    def pretile(arr):
        # [A, D, TOK] -> [NCH, P, A*DCH, CH]: one contiguous run per partition
        A = arr.shape[0]
        return np.ascontiguousarray(
            arr.reshape(A, DCH, P, TOK // CH, CH)
               .transpose(3, 2, 0, 1, 4)
               .reshape(TOK // CH, P, A * DCH, CH)
        )

    in_maps = []
    for ci in range(N_CORES):
        Hc = np.ascontiguousarray(H[ci * BL:(ci + 1) * BL].reshape(TOK, D).T)
        hhi = Hc.astype(np.float16)
        if USE_LO:
            hlo = (Hc - hhi.astype(np.float32)).astype(np.float16)
            harr = np.stack([hhi, hlo], axis=0)       # [A, D, TOK]
        else:
            harr = hhi[None]
        im = {
            "h2": pretile(harr),
            "wstk": wstk,
            "bvec": bvec,
            "mask12": mask12,
            "tmat": tmat,
        }
        if USE_F8:
            lo8 = ((Hc - hhi.astype(np.float32)) * F8_SCALE).astype(f8dt)
            im["h8"] = pretile(lo8[None])
            im["wstk8"] = wstk8
        in_maps.append(im)
    return in_maps


def _unshard(results):
    outs = []
    for ci in range(N_CORES):
        ot = results[ci]["out"]  # [2, TOK] f32
        outs.append(ot.reshape(OUT, BL, N).transpose(1, 2, 0).reshape(BL, N * OUT))
    return np.ascontiguousarray(np.concatenate(outs, axis=0).astype(np.float32))


def kernel(H, bus_type, W, b):
    nc = _build_nc()
    in_maps = _host_prep(H, bus_type, W, b)
    res = bass_utils.run_bass_kernel_spmd(
        nc, in_maps, core_ids=list(range(N_CORES))
    )
    return _unshard(res.results)


if __name__ == "__main__":
    rng = np.random.default_rng(0)
    H = rng.standard_normal((B, N, D)).astype(np.float32)
    bus_type = rng.integers(0, N_TYPES, size=(B, N)).astype(np.int64)
    W = rng.uniform(-1 / 32, 1 / 32, size=(N_TYPES, D, OUT)).astype(np.float32)
    b = rng.uniform(-1 / 32, 1 / 32, size=(N_TYPES, OUT)).astype(np.float32)
    got = kernel(H, bus_type, W, b)
    types = bus_type[0]
    want = (np.einsum("bnd,ndo->bno", H, W[types]) + b[types][None]).reshape(B, -1)
    err = np.abs(got - want)
    print("max abs err:", err.max(), "absmax-rel:", err.max() / np.abs(want).max())



# revision 6
# speedup vs baseline: 2.3395x; 2.3395x over previous
"""Trainium2 Bass kernel for nn_BusDecoder (moe_routing).

Computes out[b, n*2+o] = sum_d H[b,n,d] * W[t_n, d, o] + b[t_n, o] with
t_n = bus_type[0, n], for B=32, N=4096, D=1024, OUT=2, 3 types.

Strategy (memory-bound regime):
  - Data-parallel over batch B across 8 cores (B_local=4 per core).
  - H streams as f16 (absmax-rel err ~2e-4 vs the 2e-2 gate); W rides as a
    12-column f16 stack [Whi | Wlo] with Wlo the f32-f16 residual, so the
    einsum sees W at effectively full f32 precision for free (the mask stage
    sums both halves).
  - H is pre-tiled on the host so every DMA block is one contiguous run per
    partition (this is the main memory-rate lever).
  - Per 512-token group: 8 accumulating f16 matmuls [K=128, M=12, N=512]
    into PSUM, then one VectorE scalar_tensor_tensor applies bias + one-hot
    routing mask (by bus type) writing f16 directly, and a single f16 matmul
    with a constant 0/1 matrix T[12, 2] pair-sums into out[2, 512] in PSUM;
    the store DMAs straight from PSUM on the scalar HWDGE ring so output
    never blocks the H-load ring. The select for group g is emitted one
    group late so the PE never waits on VectorE.
"""

import os

import numpy as np

import concourse.bacc as bacc
import concourse.bass_utils as bass_utils
import concourse.mybir as mybir
import concourse.tile as tile

B, N, D, OUT = 32, 4096, 1024, 2
N_TYPES = 3
N_CORES = 8
BL = B // N_CORES          # 4 batch rows per core
TOK = BL * N               # 16384 tokens per core
P = 128
DCH = D // P               # 8 contraction chunks
G = 512                    # tokens per matmul group (one PSUM bank of fp32)
NG = TOK // G              # 32 groups
C12 = 12                   # weight stack width (Whi | Wlo)

BT = 512                   # tokens per H DMA block (host pretile granularity)

_CACHED_NC = {}


def _build_nc(repeat=1, bt=BT, hbufs=4, split_dma=False, psbufs=3,
              dual_ring=False, mode="full"):
    # repeat>1 wraps the body in a device-side For_i loop running the
    # identical workload `repeat` times — used only by test.py to measure
    # per-execution hardware time through the high-latency axon tunnel.
    # mode: "full" | "dma" (loads only) | "compute" (loads once, loops math)
    key = (repeat, bt, hbufs, split_dma, psbufs, dual_ring, mode)
    if key in _CACHED_NC:
        return _CACHED_NC[key]

    f16 = mybir.dt.float16
    f32 = mybir.dt.float32

    nc = bacc.Bacc("TRN2", debug=False)
    # h2 is host-pre-tiled to the exact per-block SBUF layout so each block
    # DMA reads one contiguous run per partition:
    #   h2[c, p, do, t] = (H^T)[do*128+p, c*BT+t]
    h2 = nc.dram_tensor("h2", [TOK // bt, P, DCH, bt], f16,
                        kind="ExternalInput")
    wstk = nc.dram_tensor("wstk", [D, C12], f16, kind="ExternalInput")
    bvec = nc.dram_tensor("bvec", [C12, 1], f32, kind="ExternalInput")
    mask = nc.dram_tensor("mask12", [C12, TOK], f16, kind="ExternalInput")
    tmat = nc.dram_tensor("tmat", [C12, OUT], f16, kind="ExternalInput")
    out = nc.dram_tensor("out", [OUT, TOK], f32, kind="ExternalOutput")

    with tile.TileContext(nc) as tc:
        with (
            tc.tile_pool(name="const", bufs=1) as cp,
            tc.tile_pool(name="hp", bufs=hbufs) as hp,
            tc.tile_pool(name="wk", bufs=3) as wk,
            tc.tile_pool(name="ps", bufs=psbufs, space="PSUM") as ps,
            tc.tile_pool(name="ps2", bufs=2, space="PSUM") as ps2,
        ):
            wt = cp.tile([P, DCH, C12], f16, name="wt")
            nc.sync.dma_start(wt[:], wstk.ap().rearrange("(do p) c -> p do c", p=P))
            bv = cp.tile([C12, 1], f32, name="bv")
            nc.sync.dma_start(bv[:], bvec.ap())
            tt = cp.tile([C12, OUT], f16, name="tt")
            nc.sync.dma_start(tt[:], tmat.ap())
            # mask rides the scalar HWDGE ring so it never delays H loads
            msk = cp.tile([C12, TOK], f16, name="msk")
            nc.scalar.dma_start(msk[:], mask.ap())

            hv = h2.ap()

            def body():
                _emit_body(nc, hv, out, hp, wk, ps, ps2, wt, bv, tt, msk,
                           bt, split_dma, dual_ring, mode)

            if repeat == 1:
                body()
            else:
                with tc.For_i(0, repeat, 1):
                    body()

    nc.compile()
    _CACHED_NC[key] = nc
    return nc


def _emit_body(nc, hv, out, hp, wk, ps, ps2, wt, bv, tt, msk,
               bt, split_dma, dual_ring, mode="full"):
    f16 = mybir.dt.float16
    f32 = mybir.dt.float32
    gpb = bt // G   # groups per DMA block

    def emit_main(ht, g):
        gs = slice(g * G, (g + 1) * G)
        p = ps.tile([C12, G], f32, name="p")
        for do in range(DCH):
            nc.tensor.matmul(
                p[:], wt[:, do, :], ht[:, do, gs],
                start=(do == 0), stop=(do == DCH - 1),
                skip_group_check=True,
            )
        return p

    def emit_select(p, off):
        # m = f16((p + bias) * mask); one pass on VectorE, then a single f16
        # pair-sum matmul with the constant 0/1 matrix lands out[2, G] in
        # PSUM, stored straight to DRAM from there.
        m = wk.tile([C12, G], f16, name="m")
        nc.vector.scalar_tensor_tensor(
            m[:], p[:], bv[:, 0:1], msk[:, off:off + G],
            mybir.AluOpType.add, mybir.AluOpType.mult,
        )
        p2 = ps2.tile([OUT, G], f32, name="p2")
        nc.tensor.matmul(
            p2[:], tt[:], m[:], start=True, stop=True, skip_group_check=True,
        )
        # PSUM -> SBUF on the (otherwise idle) Activation engine, then store
        sg = wk.tile([OUT, G], f32, name="sg")
        nc.scalar.copy(sg[:], p2[:])
        nc.scalar.dma_start(out.ap()[:, off:off + G], sg[:])

    if mode == "compute":
        ht0 = hp.tile([P, DCH, bt], f16, name="ht", bufs=1)
        nc.sync.dma_start(ht0[:], hv[0])
        pending = None
        for c in range(TOK // bt):
            for g in range(gpb):
                p = emit_main(ht0, g)
                if pending is not None:
                    emit_select(*pending)
                pending = (p, c * bt + g * G)
        emit_select(*pending)
        return

    pending = None
    for c in range(TOK // bt):
        ht = hp.tile([P, DCH, bt], f16, name="ht")
        ring = nc.gpsimd if (dual_ring and c % 2) else nc.sync
        if split_dma:
            ring.dma_start(ht[:, : DCH // 2], hv[c, :, : DCH // 2])
            ring.dma_start(ht[:, DCH // 2:], hv[c, :, DCH // 2:])
        else:
            ring.dma_start(ht[:], hv[c])
        if mode == "dma":
            # keep a reader so buffers recycle without stalling the queue
            nc.vector.tensor_copy(msk[0:1, 0:8], ht[0:1, 0, 0:8])
            continue
        for g in range(gpb):
            p = emit_main(ht, g)
            if pending is not None:
                emit_select(*pending)
            pending = (p, c * bt + g * G)
    if mode == "dma":
        return
    emit_select(*pending)


def _host_prep(H, bus_type, W, b, bt=BT):
    """Shard + precision-split inputs; returns per-core in_maps."""
    H = np.asarray(H, dtype=np.float32)
    W = np.asarray(W, dtype=np.float32)
    b = np.asarray(b, dtype=np.float32)
    types = np.asarray(bus_type)[0].astype(np.int64)  # decoder choice = row 0

    # Weight stack [D, 12]: cols 2t+o = Whi[t,:,o], cols 6+2t+o = Wlo[t,:,o]
    W6 = np.ascontiguousarray(W.transpose(1, 0, 2).reshape(D, 2 * N_TYPES))
    Whi = W6.astype(np.float16)
    Wlo = (W6 - Whi.astype(np.float32)).astype(np.float16)
    wstk = np.ascontiguousarray(np.concatenate([Whi, Wlo], axis=1))

    # Exact f32 bias, applied per-partition on VectorE before the mask-mul
    bvec = np.zeros((C12, 1), np.float32)
    bvec[0:2 * N_TYPES, 0] = b.reshape(2 * N_TYPES)

    # One-hot routing mask per token (token j = b_local*N + n -> depends on n)
    oh = (types[None, :] == np.arange(N_TYPES)[:, None])      # [3, N]
    m6 = np.repeat(oh, 2, axis=0)                             # [6, N]
    m6t = np.tile(m6, (1, BL)).astype(np.float16)             # [6, TOK]
    mask12 = np.ascontiguousarray(np.concatenate([m6t, m6t], axis=0))

    # Constant pair-sum matrix: out[o] = sum_{c: c%2==o} m[c] (exact in f16)
    tmat = np.zeros((C12, OUT), np.float16)
    tmat[0::2, 0] = 1.0
    tmat[1::2, 1] = 1.0

    def pretile(arr):
        # [D, TOK] -> [NB, P, DCH, bt]: one contiguous run per partition
        return np.ascontiguousarray(
            arr.reshape(DCH, P, TOK // bt, bt)
               .transpose(2, 1, 0, 3)
               .reshape(TOK // bt, P, DCH, bt)
        )

    in_maps = []
    for ci in range(N_CORES):
        Hc = np.ascontiguousarray(H[ci * BL:(ci + 1) * BL].reshape(TOK, D).T)
        im = {
            "h2": pretile(Hc.astype(np.float16)),
            "wstk": wstk,
            "bvec": bvec,
            "mask12": mask12,
            "tmat": tmat,
        }
        in_maps.append(im)
    return in_maps


def _unshard(results):
    outs = []
    for ci in range(N_CORES):
        ot = results[ci]["out"]  # [2, TOK] f32
        outs.append(ot.reshape(OUT, BL, N).transpose(1, 2, 0).reshape(BL, N * OUT))
    return np.ascontiguousarray(np.concatenate(outs, axis=0).astype(np.float32))


def kernel(H, bus_type, W, b):
    nc = _build_nc()
    in_maps = _host_prep(H, bus_type, W, b)
    res = bass_utils.run_bass_kernel_spmd(
        nc, in_maps, core_ids=list(range(N_CORES))
    )
    return _unshard(res.results)


if __name__ == "__main__":
    rng = np.random.default_rng(0)
    H = rng.standard_normal((B, N, D)).astype(np.float32)
    bus_type = rng.integers(0, N_TYPES, size=(B, N)).astype(np.int64)
    W = rng.uniform(-1 / 32, 1 / 32, size=(N_TYPES, D, OUT)).astype(np.float32)
    b = rng.uniform(-1 / 32, 1 / 32, size=(N_TYPES, OUT)).astype(np.float32)
    got = kernel(H, bus_type, W, b)
    types = bus_type[0]
    want = (np.einsum("bnd,ndo->bno", H, W[types]) + b[types][None]).reshape(B, -1)
    err = np.abs(got - want)
    print("max abs err:", err.max(), "absmax-rel:", err.max() / np.abs(want).max())
